# revision 25
# baseline (speedup 1.0000x reference)
"""Trainium2 Bass kernel for nn_Alignment (speaker-conditioned linear +
ragged length-regulate expansion + f0/rmse/position embedding concat).

Data-parallel over batch: 16 samples -> 8 NeuronCores x 2 samples.

Per-core device program (shapes per core):
  x [2, 512, 512], speaker/duration [2, 512] (passed as f32),
  f0/rmse/position [2, 4096], weights replicated.
  out [2, 4096, 641] f32.

Pipeline per sample:
  1. h = concat(x, emb_speaker[speaker]) @ W_lin + b_lin  on TensorE
     (x transposed via identity matmuls; speaker path as one-hot matmul
      against emb_speaker @ W_s + b_lin broadcast).
  2. cum = cumsum(duration) via tensor_tensor_scan; per-token scatter
     offsets computed on VectorE.
  3. x_expanded written by indirect scatter DMAs straight out of SBUF.
     HW DGE contract (probed on silicon; the bass interp model of
     multi-offset indirect DMA does NOT match hardware): one descriptor
     per partition, offsets [128, 1], payload = the partition's free
     extent, dest element = element_offset + offset * payload. Token
     rows are stored duplicated ([row | row], 2*641 wide) so aligned
     even frame PAIRS go out with payload 1282 (offset = frame/2) and
     the odd boundary frames with payload 641 — 5 scatters per
     128-token chunk instead of 8, cutting the ~1.6us/instr serial
     GpSimd DGE chain. Invalid slots get offset 8192 and are dropped by
     the bounds check. False WAW deps between the scatters (Tile can't
     see the dynamic rows are disjoint) are removed by hand; only
     scatter -> ext-DMA ordering within a sample is kept (scatters also
     zero the ext columns of rows they write).
  4. f0/rmse/pos columns computed on VectorE and written with one DMA
     per sample over the scattered rows.
"""

import numpy as np

import concourse.bacc as bacc
import concourse.bass as bass
import concourse.tile as tile
from concourse import mybir
from concourse.bass_utils import run_bass_kernel_spmd
from concourse.masks import make_identity

B, T, ENC2 = 16, 512, 512
SPK_N, SPK_D = 128, 64
AD = 512
FD = 64
Y = 4096
DOUT = AD + FD + FD + 1  # 641
NCORES = 8
BPC = B // NCORES  # samples per core
P = 128
TC = T // P  # 4 token chunks
CC = ENC2 // P  # 4 x-channel chunks
YC = Y // P  # 32
REP = 8  # max duration (duration in [0, 8))
BIG = 8192.0  # OOB marker for dropped scatter slots

F32 = mybir.dt.float32
BF16 = mybir.dt.bfloat16
I32 = mybir.dt.int32

# Rows past sum(duration) are left untouched: run_bass_kernel_spmd
# pre-zeros ExternalOutput buffers (native path memsets; the axon/PJRT
# path donates zero-filled buffers) — verified empirically on this stack.

_CACHE = {}


def _build_nc(en_scatter1=True, en_scatter2=True, en_scan=True, en_ext=True,
              en_h=True):
    nc = bacc.Bacc("TRN2", target_bir_lowering=False, debug=False,
                   num_devices=NCORES)

    x_d = nc.dram_tensor("x", [BPC, T, ENC2], F32, kind="ExternalInput")
    spk_d = nc.dram_tensor("speaker_f", [BPC, T], F32, kind="ExternalInput")
    dur_d = nc.dram_tensor("duration_f", [BPC, T], F32, kind="ExternalInput")
    f0_d = nc.dram_tensor("f0", [BPC, Y], F32, kind="ExternalInput")
    rmse_d = nc.dram_tensor("rmse", [BPC, Y], F32, kind="ExternalInput")
    pos_d = nc.dram_tensor("position", [BPC, Y], F32, kind="ExternalInput")
    emb_d = nc.dram_tensor("emb_speaker", [SPK_N, SPK_D], F32, kind="ExternalInput")
    wlin_d = nc.dram_tensor("W_lin", [ENC2 + SPK_D, AD], F32, kind="ExternalInput")
    blin_d = nc.dram_tensor("b_lin", [AD], F32, kind="ExternalInput")
    wf0_d = nc.dram_tensor("W_f0", [1, FD], F32, kind="ExternalInput")
    bf0_d = nc.dram_tensor("b_f0", [FD], F32, kind="ExternalInput")
    wrm_d = nc.dram_tensor("W_rmse", [1, FD], F32, kind="ExternalInput")
    brm_d = nc.dram_tensor("b_rmse", [FD], F32, kind="ExternalInput")
    out_d = nc.dram_tensor("out", [BPC, Y, DOUT], F32, kind="ExternalOutput")

    out_flat = out_d.ap().flatten_outer_dims()  # [BPC*Y, DOUT] dense
    out_pair = out_flat.rearrange("(a b) d -> a (b d)", b=2)  # [BPC*Y/2, 2*DOUT]

    from contextlib import ExitStack
    with tile.TileContext(nc) as tc, ExitStack() as ctx:
        const = ctx.enter_context(tc.tile_pool(name="const", bufs=1))
        sb = ctx.enter_context(tc.tile_pool(name="sb", bufs=2))
        ps = ctx.enter_context(tc.tile_pool(name="ps", bufs=2, space="PSUM"))
        ps1 = ctx.enter_context(tc.tile_pool(name="ps1", bufs=1, space="PSUM"))

        # out-writing DMA instructions, for false-WAW removal below:
        # list of (instruction, sample, kind)
        out_writers = []

        # ---------------- constants / weights ----------------
        ident = const.tile([P, P], F32)
        make_identity(nc, ident[:])

        ones_row = const.tile([1, P], F32)
        nc.gpsimd.memset(ones_row[:], 1.0)

        ones_pp = const.tile([P, P], F32)
        nc.gpsimd.memset(ones_pp[:], 1.0)

        iota_p_i = const.tile([P, 1], I32)
        nc.gpsimd.iota(iota_p_i[:], pattern=[[0, 1]], base=0, channel_multiplier=1)
        iota_p = const.tile([P, 1], F32)
        nc.gpsimd.tensor_copy(iota_p[:], iota_p_i[:])

        # pair index k = 0..2 per chunk, and its validity threshold 2k+2
        NPAIR = 3
        iota_k_i = const.tile([P, TC * NPAIR], I32)
        nc.gpsimd.iota(iota_k_i[:], pattern=[[0, TC], [1, NPAIR]], base=0,
                       channel_multiplier=0)
        iota_k = const.tile([P, TC * NPAIR], F32)
        nc.gpsimd.tensor_copy(iota_k[:], iota_k_i[:])

        # W_lin x-part chunks [P, AD] each
        wx = []
        for cc in range(CC):
            t = const.tile([P, AD], F32, tag=f"wx{cc}")
            nc.scalar.dma_start(out=t[:], in_=wlin_d.ap()[cc * P:(cc + 1) * P, :])
            wx.append(t)
        ws = const.tile([SPK_D, AD], F32)
        nc.scalar.dma_start(out=ws[:], in_=wlin_d.ap()[ENC2:ENC2 + SPK_D, :])
        blin = const.tile([1, AD], F32)
        nc.scalar.dma_start(out=blin[:], in_=blin_d.ap().unsqueeze(0))

        # emb_speaker -> embT [SPK_D, SPK_N]
        emb_sb = sb.tile([SPK_N, SPK_D], F32)
        nc.scalar.dma_start(out=emb_sb[:], in_=emb_d.ap())
        embT_ps = ps1.tile([SPK_D, SPK_N], F32, tag="setup")
        nc.tensor.transpose(out=embT_ps[:], in_=emb_sb[:], identity=ident[:])
        embT = const.tile([SPK_D, SPK_N], F32)
        nc.vector.tensor_copy(embT[:], embT_ps[:])

        # wspk[s, :] = emb_speaker[s] @ W_s + b_lin   [SPK_N, AD]
        wspk_ps = ps1.tile([SPK_N, AD], F32, tag="setup")
        nc.tensor.matmul(out=wspk_ps[:], lhsT=embT[:], rhs=ws[:],
                         start=True, stop=False)
        nc.tensor.matmul(out=wspk_ps[:], lhsT=ones_row[:], rhs=blin[:],
                         start=False, stop=True)
        wspk = const.tile([SPK_N, AD], F32)
        nc.vector.tensor_copy(wspk[:], wspk_ps[:])

        # f0/rmse weight+bias rows broadcast to 128 partitions:
        # cols [0:64]=W_f0 [64:128]=b_f0 [128:192]=W_rmse [192:256]=b_rmse
        fbrow = sb.tile([1, 4 * FD], F32)
        nc.scalar.dma_start(out=fbrow[:, 0:FD], in_=wf0_d.ap())
        nc.scalar.dma_start(out=fbrow[:, FD:2 * FD], in_=bf0_d.ap().unsqueeze(0))
        nc.scalar.dma_start(out=fbrow[:, 2 * FD:3 * FD], in_=wrm_d.ap())
        nc.scalar.dma_start(out=fbrow[:, 3 * FD:4 * FD], in_=brm_d.ap().unsqueeze(0))
        wfb_ps = ps1.tile([P, 4 * FD], F32, tag="setup")
        nc.tensor.matmul(out=wfb_ps[:], lhsT=ones_row[:], rhs=fbrow[:],
                         start=True, stop=True)
        wfb = const.tile([P, 4 * FD], F32)
        nc.vector.tensor_copy(wfb[:], wfb_ps[:])

        # ---------------- duration scan (both samples at once) -----------
        durrows = sb.tile([BPC, T], F32)
        cumrows = sb.tile([BPC, T], F32)
        zrows = sb.tile([BPC, T], F32)
        nc.gpsimd.memset(zrows[:], 0.0)
        nc.sync.dma_start(out=durrows[:], in_=dur_d.ap())
        if en_scan:
            nc.vector.tensor_tensor_scan(
                out=cumrows[:], data0=durrows[:], data1=zrows[:],
                initial=0.0, op0=mybir.AluOpType.add, op1=mybir.AluOpType.add)
        else:
            nc.vector.tensor_copy(cumrows[:], durrows[:])

        # transpose -> ct_all [P, TC*4]:
        # ct_all[p, c*4+j]: j in {0,1}: cum sample j at token c*128+p;
        #                   j in {2,3}: duration sample j-2.
        W4 = 2 * BPC
        ct_ps = ps1.tile([P, TC * W4], F32)
        for c in range(TC):
            nc.tensor.transpose(out=ct_ps[:, c * W4:c * W4 + BPC],
                                in_=cumrows[:, c * P:(c + 1) * P],
                                identity=ident[0:BPC, 0:BPC])
            nc.tensor.transpose(out=ct_ps[:, c * W4 + BPC:(c + 1) * W4],
                                in_=durrows[:, c * P:(c + 1) * P],
                                identity=ident[0:BPC, 0:BPC])
        ct_all = sb.tile([P, TC * W4], F32)
        nc.vector.tensor_copy(ct_all[:], ct_ps[:])
        ct_v = ct_all[:].rearrange("p (c j) -> p c j", j=W4)

        for b in range(BPC):
            # ---------------- scatter offsets ----------------
            # frames of token i: [s, e). Aligned (even-start) frame PAIRS
            # are written with 1282-element payloads (coef = 2*DOUT), the
            # odd boundary frames with single-row payloads.
            cum_t = ct_v[:, :, b]            # [P, TC] e = cum at token
            dur_t = ct_v[:, :, BPC + b]      # [P, TC]
            cpt = sb.tile([P, TC], F32, tag="cpt")  # s = cum - dur
            nc.gpsimd.tensor_tensor(out=cpt[:], in0=cum_t, in1=dur_t,
                                    op=mybir.AluOpType.subtract)
            # parities via int32 and-1 (values are exact integers in f32)
            se_i = sb.tile([P, 2 * TC], I32, tag="se_i")
            nc.gpsimd.tensor_copy(se_i[:, 0:TC], cpt[:])
            nc.gpsimd.tensor_copy(se_i[:, TC:2 * TC], cum_t)
            par_i = sb.tile([P, 2 * TC], I32, tag="par_i")
            nc.gpsimd.tensor_scalar(out=par_i[:], in0=se_i[:], scalar1=1,
                                    scalar2=None,
                                    op0=mybir.AluOpType.bitwise_and)
            par = sb.tile([P, 2 * TC], F32, tag="par")
            nc.gpsimd.tensor_copy(par[:], par_i[:])
            s_par = par[:, 0:TC]
            e_par = par[:, TC:2 * TC]

            # a_half = (s + s_par)/2 ; d = e - (s + s_par)
            ah = sb.tile([P, TC], F32, tag="ah")
            nc.gpsimd.tensor_tensor(out=ah[:], in0=cpt[:], in1=s_par,
                                    op=mybir.AluOpType.add)
            dtile = sb.tile([P, TC], F32, tag="dtile")
            nc.gpsimd.tensor_tensor(out=dtile[:], in0=cum_t, in1=ah[:],
                                    op=mybir.AluOpType.subtract)
            nc.gpsimd.tensor_scalar(out=ah[:], in0=ah[:], scalar1=0.5,
                                    scalar2=None, op0=mybir.AluOpType.mult)

            # pair offsets [P, TC, NPAIR]: ah + k, invalid when d < 2k+2
            pf = sb.tile([P, TC * NPAIR], F32, tag="pf")
            nc.gpsimd.tensor_tensor(
                out=pf[:],
                in0=ah[:].unsqueeze(2).to_broadcast([P, TC, NPAIR]),
                in1=iota_k[:].rearrange("p (c k) -> p c k", k=NPAIR),
                op=mybir.AluOpType.add)
            thr = sb.tile([P, TC * NPAIR], F32, tag="thr")
            nc.gpsimd.tensor_scalar(out=thr[:], in0=iota_k[:], scalar1=2.0,
                                    scalar2=2.0, op0=mybir.AluOpType.mult,
                                    op1=mybir.AluOpType.add)
            pinv = sb.tile([P, TC * NPAIR], F32, tag="pinv")
            nc.gpsimd.tensor_tensor(
                out=pinv[:],
                in0=dtile[:].unsqueeze(2).to_broadcast([P, TC, NPAIR]),
                in1=thr[:].rearrange("p (c k) -> p c k", k=NPAIR),
                op=mybir.AluOpType.is_lt)
            nc.gpsimd.scalar_tensor_tensor(
                out=pf[:], in0=pinv[:], scalar=BIG, in1=pf[:],
                op0=mybir.AluOpType.mult, op1=mybir.AluOpType.add)
            offp = sb.tile([P, TC * NPAIR], I32, tag="offp")
            nc.gpsimd.tensor_copy(offp[:], pf[:])

            # single-start: s, valid iff s odd and dur >= 1
            # single-end: e-1, valid iff e odd and dur >= 1
            sgl = sb.tile([P, 2 * TC], F32, tag="sgl")
            m = sb.tile([P, 2 * TC], F32, tag="m")
            nc.gpsimd.tensor_tensor(out=m[:, 0:TC], in0=s_par, in1=dur_t,
                                    op=mybir.AluOpType.min)
            nc.gpsimd.tensor_tensor(out=m[:, TC:2 * TC], in0=e_par, in1=dur_t,
                                    op=mybir.AluOpType.min)
            nc.gpsimd.tensor_scalar(out=m[:], in0=m[:], scalar1=0.5,
                                    scalar2=None, op0=mybir.AluOpType.is_lt)
            nc.gpsimd.tensor_copy(sgl[:, 0:TC], cpt[:])
            nc.gpsimd.tensor_scalar(out=sgl[:, TC:2 * TC], in0=cum_t,
                                    scalar1=1.0, scalar2=None,
                                    op0=mybir.AluOpType.subtract)
            nc.gpsimd.scalar_tensor_tensor(
                out=sgl[:], in0=m[:], scalar=BIG, in1=sgl[:],
                op0=mybir.AluOpType.mult, op1=mybir.AluOpType.add)
            offsg = sb.tile([P, 2 * TC], I32, tag="offsg")
            nc.gpsimd.tensor_copy(offsg[:], sgl[:])

            # ---------------- speaker one-hot ----------------
            spkrow = sb.tile([1, T], F32, tag="spkrow")
            nc.sync.dma_start(out=spkrow[:], in_=spk_d.ap()[b:b + 1, :])
            spkb_ps = ps.tile([P, T], F32, tag="mm_small")
            nc.tensor.matmul(out=spkb_ps[:], lhsT=ones_row[:], rhs=spkrow[:],
                             start=True, stop=True)
            sspk = sb.tile([P, T], F32, tag="sspk")
            nc.vector.tensor_scalar(out=sspk[:], in0=spkb_ps[:],
                                    scalar1=iota_p[:], scalar2=None,
                                    op0=mybir.AluOpType.is_equal)

            # ---------------- x load + transpose (per token chunk) -----
            # For token chunk tc, the 4 stationaries are transposes of
            # x_sb[tc][:, cc*128:(cc+1)*128] -- all from ONE x tile, so
            # each h chunk flows independently (h[0] is ready ~15us
            # earlier than with the per-cc grouping).
            xts = []
            xTts = []
            for tcc in range(TC):
                xt = sb.tile([P, ENC2], F32, tag=f"x{tcc}")
                nc.sync.dma_start(out=xt[:],
                                  in_=x_d.ap()[b, tcc * P:(tcc + 1) * P, :])
                xts.append(xt)
                xTt_ps = ps.tile([P, T], F32, tag="xTps")
                for cc in range(CC):
                    nc.tensor.transpose(
                        out=xTt_ps[:, cc * P:(cc + 1) * P],
                        in_=xt[:, cc * P:(cc + 1) * P],
                        identity=ident[:])
                xTt = sb.tile([P, T], F32, tag=f"xTt{tcc}")
                if tcc % 2 == 0:
                    nc.vector.tensor_copy(xTt[:], xTt_ps[:])
                else:
                    nc.scalar.copy(xTt[:], xTt_ps[:])
                xTts.append(xTt)

            # ---------------- h = concat(x, spk) @ W + b ----------------
            # h2_sb[p, tc, :] = token row DUPLICATED: [row | row], each
            # 641 wide with cols [AD:DOUT] zeroed (ext DMA overwrites
            # those later). Pair scatters use the full 1282 elements,
            # single scatters the first 641.
            h2_sb = sb.tile([P, TC * 2 * DOUT], F32, tag="h")
            h_v = h2_sb[:].rearrange("p (c d) -> p c d", c=TC)
            nc.gpsimd.memset(h_v[:, :, AD:DOUT], 0.0)
            nc.gpsimd.memset(h_v[:, :, DOUT + AD:2 * DOUT], 0.0)
            for tcc in range(TC):
                h_ps = ps.tile([P, AD], F32, tag="hps")
                for cc in range(CC):
                    nc.tensor.matmul(out=h_ps[:],
                                     lhsT=xTts[tcc][:, cc * P:(cc + 1) * P],
                                     rhs=wx[cc][:],
                                     start=(cc == 0), stop=False)
                nc.tensor.matmul(out=h_ps[:],
                                 lhsT=sspk[:, tcc * P:(tcc + 1) * P],
                                 rhs=wspk[:],
                                 start=False, stop=True)
                nc.scalar.copy(h_v[:, tcc, 0:AD], h_ps[:])
                nc.vector.tensor_copy(h_v[:, tcc, DOUT:DOUT + AD], h_ps[:])

            # ---------------- x_expanded scatter ----------------
            # HW contract (probed): offsets must be [P, 1] (one descriptor
            # per partition), payload = the partition's 2-D free extent,
            # dest element = element_offset + offset * payload.
            # Pair scatters: payload 2*DOUT, offset = even-frame/2.
            # Single scatters: payload DOUT, offset = frame.
            if en_scatter1:
                for c in range(TC):
                    for k in range(NPAIR):
                        sc = nc.gpsimd.indirect_dma_start(
                            out=out_pair,
                            out_offset=bass.IndirectOffsetOnAxis(
                                ap=offp[:, c * NPAIR + k:c * NPAIR + k + 1],
                                axis=0),
                            in_=h_v[:, c, :],
                            in_offset=None,
                            element_offset=b * Y * DOUT,
                            bounds_check=Y // 2 - 1,
                            oob_is_err=False)
                        out_writers.append((sc.ins, b, "scatter"))
                    for which in (0, 1):
                        sc = nc.gpsimd.indirect_dma_start(
                            out=out_flat,
                            out_offset=bass.IndirectOffsetOnAxis(
                                ap=offsg[:, which * TC + c:which * TC + c + 1],
                                axis=0),
                            in_=h_v[:, c, 0:DOUT],
                            in_offset=None,
                            element_offset=b * Y * DOUT,
                            bounds_check=Y - 1,
                            oob_is_err=False)
                        out_writers.append((sc.ins, b, "scatter"))

            # ---------------- f0 / rmse / position columns ----------------
            # layout [P, YC, FD+FD+1], frame t = p*YC + c
            f0_sb = sb.tile([P, YC], F32, tag="f0sb")
            nc.sync.dma_start(out=f0_sb[:],
                              in_=f0_d.ap()[b].rearrange("(p c) -> p c", p=P))
            rm_sb = sb.tile([P, YC], F32, tag="rmsb")
            nc.sync.dma_start(out=rm_sb[:],
                              in_=rmse_d.ap()[b].rearrange("(p c) -> p c", p=P))
            po_sb = sb.tile([P, YC], F32, tag="posb")
            nc.sync.dma_start(out=po_sb[:],
                              in_=pos_d.ap()[b].rearrange("(p c) -> p c", p=P))

            EW = 2 * FD + 1  # 129
            ext = sb.tile([P, YC * EW], F32, tag="ext")
            ext_v = ext[:].rearrange("p (c d) -> p c d", d=EW)
            nc.vector.tensor_tensor(
                out=ext_v[:, :, 0:FD],
                in0=f0_sb[:].unsqueeze(2).to_broadcast([P, YC, FD]),
                in1=wfb[:, 0:FD].unsqueeze(1).to_broadcast([P, YC, FD]),
                op=mybir.AluOpType.mult)
            nc.vector.tensor_tensor(
                out=ext_v[:, :, 0:FD],
                in0=ext_v[:, :, 0:FD],
                in1=wfb[:, FD:2 * FD].unsqueeze(1).to_broadcast([P, YC, FD]),
                op=mybir.AluOpType.add)
            nc.vector.tensor_tensor(
                out=ext_v[:, :, FD:2 * FD],
                in0=rm_sb[:].unsqueeze(2).to_broadcast([P, YC, FD]),
                in1=wfb[:, 2 * FD:3 * FD].unsqueeze(1).to_broadcast([P, YC, FD]),
                op=mybir.AluOpType.mult)
            nc.vector.tensor_tensor(
                out=ext_v[:, :, FD:2 * FD],
                in0=ext_v[:, :, FD:2 * FD],
                in1=wfb[:, 3 * FD:4 * FD].unsqueeze(1).to_broadcast([P, YC, FD]),
                op=mybir.AluOpType.add)
            nc.vector.tensor_copy(ext_v[:, :, 2 * FD:EW],
                                  po_sb[:].unsqueeze(2))
            ext_out = out_d.ap()[b].rearrange("(p c) d -> p c d", p=P)
            e1 = nc.sync.dma_start(out=ext_out[:, 0:YC // 2, AD:DOUT],
                                   in_=ext_v[:, 0:YC // 2, :])
            e2 = nc.scalar.dma_start(out=ext_out[:, YC // 2:YC, AD:DOUT],
                                     in_=ext_v[:, YC // 2:YC, :])
            out_writers.append((e1.ins, b, "ext"))
            out_writers.append((e2.ins, b, "ext"))

        # ---------------- false-WAW removal ----------------
        # All scatters write disjoint output rows (disjoint (chunk, rep)
        # slots; samples use disjoint element_offset ranges), so the only
        # REAL write-order constraint among out-writers is scatter -> ext
        # within one sample (scatters also write zeros into the ext
        # columns of the rows they touch). Tile can't see this through
        # the dynamic APs and chains every pair; drop the false edges.
        for i, (ins_a, sa, ka) in enumerate(out_writers):
            for ins_b, sb_, kb in out_writers[i + 1:]:
                real = (sa == sb_) and (ka != kb)
                if not real:
                    ins_b.try_remove_dependency(ins_a.name)

    nc.compile()
    return nc


def _shard_inputs(inputs):
    x = np.ascontiguousarray(np.asarray(inputs["x"], dtype=np.float32))
    spk = np.asarray(inputs["speaker"]).astype(np.float32)
    dur = np.asarray(inputs["duration"]).astype(np.float32)
    f0 = np.ascontiguousarray(np.asarray(inputs["f0"], dtype=np.float32))
    rmse = np.ascontiguousarray(np.asarray(inputs["rmse"], dtype=np.float32))
    pos = np.ascontiguousarray(np.asarray(inputs["position"], dtype=np.float32))
    emb = np.ascontiguousarray(np.asarray(inputs["emb_speaker"], dtype=np.float32))
    wlin = np.ascontiguousarray(np.asarray(inputs["W_lin"], dtype=np.float32))
    blin = np.ascontiguousarray(np.asarray(inputs["b_lin"], dtype=np.float32))
    wf0 = np.ascontiguousarray(np.asarray(inputs["W_f0"], dtype=np.float32))
    bf0 = np.ascontiguousarray(np.asarray(inputs["b_f0"], dtype=np.float32))
    wrm = np.ascontiguousarray(np.asarray(inputs["W_rmse"], dtype=np.float32))
    brm = np.ascontiguousarray(np.asarray(inputs["b_rmse"], dtype=np.float32))

    in_maps = []
    for i in range(NCORES):
        s = slice(i * BPC, (i + 1) * BPC)
        in_maps.append({
            "x": x[s], "speaker_f": spk[s], "duration_f": dur[s],
            "f0": f0[s], "rmse": rmse[s], "position": pos[s],
            "emb_speaker": emb, "W_lin": wlin, "b_lin": blin,
            "W_f0": wf0, "b_f0": bf0, "W_rmse": wrm, "b_rmse": brm,
        })
    return in_maps


def run(inputs, trace=False):
    if "nc" not in _CACHE:
        _CACHE["nc"] = _build_nc()
    nc = _CACHE["nc"]
    in_maps = _shard_inputs(inputs)
    res = run_bass_kernel_spmd(nc, in_maps, core_ids=list(range(NCORES)),
                               trace=trace)
    out = np.concatenate([np.asarray(r["out"]) for r in res.results], axis=0)
    return out, res


def kernel(**inputs):
    out, _ = run(inputs, trace=False)
    return out
